# revision 1
# baseline (speedup 1.0000x reference)
"""Trainium2 Bass kernel for multi-head attention (B=4, T=S=1024, E=1024, H=16).

Sharding: 8 cores; core c handles batch b=c//2 and head-group g=c%2 (8 heads,
Megatron-style column split of Wq/Wk/Wv, row split of Wo). Each core returns
its 8 heads' attention probabilities and a partial out-projection; the host
sums the two partials per batch and adds the bias.

Per-core dataflow (all on one NeuronCore, scheduled by Tile):
  1. PE-transpose target/source embeddings (f32) -> embT [e, seq].
  2. Q^T = Wq^T-chunks @ TembT, K^T likewise, V in natural [s, d] layout
     (cast to bf16 for the PV matmul).
  3. Per (head, 128-row tile of T): scores = Q_h^T.T @ K_h^T accumulated in
     PSUM, merged mask added via identity-lhsT matmul accumulation; ACT Exp
     with fused row-sum (accum_out); DVE normalizes into f32 (attn output)
     and bf16 (PV operand). No max-subtraction: scores are bounded (~N(0,1)
     + mask in [0,1]) so exp() is safe in f32; verified against reference.
  4. bf16 attn tiles are transposed 128x128 via the DMA xbar (SBUF->SBUF)
     to give attnT [s, t] for the PV contraction over s.
  5. O^T = V-chunks.T @ attnT per head; out-projection from O^T chunks
     against Wo row-slice; partial result [T, E] DMA'd out.
"""

import math

import numpy as np

import concourse.bass as bass
import concourse.mybir as mybir
import concourse.tile as tile
from concourse import bacc, bass_utils

B, T, S, E = 4, 1024, 1024, 1024
H, D = 16, 64
HG, DG = 8, 512  # heads / head-dims per core
P = 128
N_CORES = 8
F32 = mybir.dt.float32
BF16 = mybir.dt.bfloat16
AF = mybir.ActivationFunctionType

ET, TT, ST = E // P, T // P, S // P  # 8, 8, 8

ts = bass.ts

# toggled by test.py to capture a hardware profile
TRACE = False
LAST_RESULTS = None


def _build(nc):
    temb = nc.dram_tensor("temb", [T, E], F32, kind="ExternalInput").ap()
    semb = nc.dram_tensor("semb", [S, E], F32, kind="ExternalInput").ap()
    wq = nc.dram_tensor("wq", [E, DG], F32, kind="ExternalInput").ap()
    wk = nc.dram_tensor("wk", [E, DG], F32, kind="ExternalInput").ap()
    wv = nc.dram_tensor("wv", [E, DG], F32, kind="ExternalInput").ap()
    wo = nc.dram_tensor("wo", [DG, E], F32, kind="ExternalInput").ap()
    msk = nc.dram_tensor("msk", [T, S], F32, kind="ExternalInput").ap()
    ident = nc.dram_tensor("ident", [P, P], F32, kind="ExternalInput").ap()
    attn_out = nc.dram_tensor("attn_out", [HG, T, S], F32, kind="ExternalOutput").ap()
    out_part = nc.dram_tensor("out_part", [T, E], F32, kind="ExternalOutput").ap()

    with tile.TileContext(nc) as tc:
        with (
            tc.tile_pool(name="const", bufs=1) as constp,
            tc.tile_pool(name="persist", bufs=1) as persist,
        ):
            id_sb = constp.tile([P, P], F32)
            nc.sync.dma_start(id_sb[:], ident)

            msk_sb = persist.tile([P, TT, S], F32)  # [p, tt, s]
            nc.sync.dma_start(msk_sb[:], msk.rearrange("(tt p) s -> p tt s", p=P))
            wo_sb = persist.tile([P, DG // P, E], F32)  # [p, chunk, e]
            nc.sync.dma_start(wo_sb[:], wo.rearrange("(c p) e -> p c e", p=P))

            qT_sb = persist.tile([P, DG // P, T], F32)  # [p, dc, t]
            kT_sb = persist.tile([P, DG // P, S], F32)
            vb_sb = persist.tile([P, ST, DG], BF16)  # [p, sc, d]
            oT_sb = persist.tile([P, DG // P, T], F32)  # [p, dc, t]

            # ---------------- phase A: embedding transpose + projections ----
            with (
                tc.tile_pool(name="pa_emb", bufs=1) as pae,
                tc.tile_pool(name="pa_w", bufs=2) as paw,
                tc.tile_pool(name="pa_rows", bufs=1) as par,
                tc.tile_pool(name="pa_ps", bufs=2, space="PSUM") as pap,
                tc.tile_pool(name="pa_ps2", bufs=2, space="PSUM") as pap2,
            ):

                def transpose_emb(src, name):
                    embT = pae.tile([P, ET, T], F32, tag="embT", name=name)
                    for t4 in range(TT // 4):
                        rows = []
                        for j in range(4):
                            row = par.tile([P, E], F32, tag="emb_row", bufs=6,
                                           name=f"row_{name}_{t4}_{j}")
                            nc.sync.dma_start(row[:], src[ts(t4 * 4 + j, P), :])
                            rows.append(row)
                        for eo in range(ET):
                            ps = pap.tile([P, 4 * P], F32, tag="tr_ps", name="tr_ps")
                            for j in range(4):
                                nc.tensor.transpose(
                                    ps[:, ts(j, P)], rows[j][:, ts(eo, P)], id_sb[:]
                                )
                            nc.scalar.copy(embT[:, eo, ts(t4, 4 * P)], ps[:])
                    return embT

                def load_w(src, name):
                    w_sb = paw.tile([P, ET, DG], F32, tag="w", name=name)
                    nc.sync.dma_start(w_sb[:], src.rearrange("(eo p) d -> p eo d", p=P))
                    return w_sb

                # Q^T from target embedding
                tembT = transpose_emb(temb, "tembT")
                wq_sb = load_w(wq, "wq_sb")
                for dc in range(DG // P):
                    for th in range(T // 512):
                        ps = pap2.tile([P, 512], F32, tag="proj_ps", name="proj_ps")
                        for eo in range(ET):
                            nc.tensor.matmul(
                                ps[:],
                                wq_sb[:, eo, ts(dc, P)],
                                tembT[:, eo, ts(th, 512)],
                                start=(eo == 0),
                                stop=(eo == ET - 1),
                            )
                        nc.scalar.copy(qT_sb[:, dc, ts(th, 512)], ps[:])

                # K^T and V from source embedding
                sembT = transpose_emb(semb, "sembT")
                wk_sb = load_w(wk, "wk_sb")
                for dc in range(DG // P):
                    for sh in range(S // 512):
                        ps = pap2.tile([P, 512], F32, tag="proj_ps", name="proj_ps")
                        for eo in range(ET):
                            nc.tensor.matmul(
                                ps[:],
                                wk_sb[:, eo, ts(dc, P)],
                                sembT[:, eo, ts(sh, 512)],
                                start=(eo == 0),
                                stop=(eo == ET - 1),
                            )
                        nc.scalar.copy(kT_sb[:, dc, ts(sh, 512)], ps[:])

                wv_sb = load_w(wv, "wv_sb")
                for sc in range(ST):
                    ps = pap2.tile([P, DG], F32, tag="proj_ps", name="proj_ps")
                    for eo in range(ET):
                        nc.tensor.matmul(
                            ps[:],
                            sembT[:, eo, ts(sc, P)],
                            wv_sb[:, eo, :],
                            start=(eo == 0),
                            stop=(eo == ET - 1),
                        )
                    nc.scalar.copy(vb_sb[:, sc, :], ps[:])  # f32 -> bf16 cast

            # ---------------- phase B: attention per head ----
            with (
                tc.tile_pool(name="pb_sbuf", bufs=3) as pb,
                tc.tile_pool(name="pb_at", bufs=2) as pat,
                tc.tile_pool(name="pb_ps", bufs=2, space="PSUM") as pbp,
                tc.tile_pool(name="pb_ops", bufs=2, space="PSUM") as pbo,
            ):
                for h in range(HG):
                    hp = 64 * (h % 2)
                    dc = h // 2
                    aT = pat.tile([P, ST, T], BF16, tag="aT", name=f"aT_{h}")
                    for tt in range(TT):
                        ps = pbp.tile([P, S], F32, tag="score_ps", name="score_ps")
                        for sc in range(S // 512):
                            nc.tensor.matmul(
                                ps[:, ts(sc, 512)],
                                qT_sb[hp : hp + 64, dc, ts(tt, P)],
                                kT_sb[hp : hp + 64, dc, ts(sc, 512)],
                                start=True,
                                stop=False,
                            )
                            nc.tensor.matmul(
                                ps[:, ts(sc, 512)],
                                id_sb[:],
                                msk_sb[:, tt, ts(sc, 512)],
                                start=False,
                                stop=True,
                            )
                        ex = pb.tile([P, S], F32, tag="ex", name="ex")
                        sums = pb.tile([P, 1], F32, tag="sums", name="sums")
                        nc.scalar.activation(ex[:], ps[:], AF.Exp, accum_out=sums[:])
                        rec = pb.tile([P, 1], F32, tag="rec", name="rec")
                        nc.vector.reciprocal(rec[:], sums[:])
                        af = pb.tile([P, S], F32, tag="af", name="af")
                        nc.vector.tensor_scalar_mul(af[:], ex[:], rec[:])
                        nc.sync.dma_start(attn_out[h, ts(tt, P), :], af[:])
                        ab = pb.tile([P, S], BF16, tag="ab", name="ab")
                        nc.vector.tensor_scalar_mul(ab[:], ex[:], rec[:])
                        for sc in range(ST):
                            nc.sync.dma_start(
                                aT[:, sc, ts(tt, P)], ab[:, ts(sc, P)], transpose=True
                            )
                    for th in range(T // 512):
                        op = pbo.tile([64, 512], F32, tag="o_ps", name="o_ps")
                        for sc in range(ST):
                            nc.tensor.matmul(
                                op[:],
                                vb_sb[:, sc, ts(h, 64)],
                                aT[:, sc, ts(th, 512)],
                                start=(sc == 0),
                                stop=(sc == ST - 1),
                            )
                        nc.vector.tensor_copy(oT_sb[hp : hp + 64, dc, ts(th, 512)], op[:])

                # ---------------- phase C: out projection ----
                for tt in range(TT):
                    for eh in range(E // 512):
                        ps = pbp.tile([P, 512], F32, tag="out_ps", name="out_ps")
                        for c in range(DG // P):
                            nc.tensor.matmul(
                                ps[:],
                                oT_sb[:, c, ts(tt, P)],
                                wo_sb[:, c, ts(eh, 512)],
                                start=(c == 0),
                                stop=(c == DG // P - 1),
                            )
                        ou = pb.tile([P, 512], F32, tag="ou", name="ou")
                        nc.scalar.copy(ou[:], ps[:])
                        nc.sync.dma_start(out_part[ts(tt, P), ts(eh, 512)], ou[:])

    return nc


_compiled = None


def _get_compiled():
    global _compiled
    if _compiled is None:
        nc = bacc.Bacc("TRN2", debug=False)
        _build(nc)
        nc.compile()
        _compiled = nc
    return _compiled


def kernel(**inputs):
    global LAST_RESULTS
    tgt = np.asarray(inputs["target_emb"], dtype=np.float32)
    src = np.asarray(inputs["source_emb"], dtype=np.float32)
    tpad = np.asarray(inputs["target_pad_mask"], dtype=np.float32)
    spad = np.asarray(inputs["source_pad_mask"], dtype=np.float32)
    amask = np.asarray(inputs["attention_mask"], dtype=np.float32)
    causal = int(np.asarray(inputs["is_casual"]))
    Wq = np.asarray(inputs["Wq"], dtype=np.float32)
    Wk = np.asarray(inputs["Wk"], dtype=np.float32)
    Wv = np.asarray(inputs["Wv"], dtype=np.float32)
    Wo = np.asarray(inputs["Wo"], dtype=np.float32)
    bo = np.asarray(inputs["bo"], dtype=np.float32)

    mask = tpad[:, :, None] * spad[:, None, :] * amask  # [B, T, S]
    if causal:
        mask = mask * np.tril(np.ones((T, S), dtype=np.float32))
    mask = np.ascontiguousarray(mask, dtype=np.float32)

    ident = np.eye(P, dtype=np.float32)
    scale = np.float32(1.0 / math.sqrt(float(D)))

    nc = _get_compiled()
    in_maps = []
    for c in range(N_CORES):
        b, g = divmod(c, 2)
        in_maps.append(
            {
                "temb": np.ascontiguousarray(tgt[b]),
                "semb": np.ascontiguousarray(src[b]),
                "wq": np.ascontiguousarray(Wq[:, g * DG : (g + 1) * DG] * scale),
                "wk": np.ascontiguousarray(Wk[:, g * DG : (g + 1) * DG]),
                "wv": np.ascontiguousarray(Wv[:, g * DG : (g + 1) * DG]),
                "wo": np.ascontiguousarray(Wo[g * DG : (g + 1) * DG, :]),
                "msk": mask[b],
                "ident": ident,
            }
        )

    results = bass_utils.run_bass_kernel_spmd(
        nc, in_maps, core_ids=list(range(N_CORES)), trace=TRACE
    )
    LAST_RESULTS = results
    res = results.results

    attn = np.empty((B, H, T, S), dtype=np.float32)
    out = np.empty((T, B, E), dtype=np.float32)
    for c in range(N_CORES):
        b, g = divmod(c, 2)
        attn[b, g * HG : (g + 1) * HG] = res[c]["attn_out"]
    for b in range(B):
        out[:, b, :] = res[2 * b]["out_part"] + res[2 * b + 1]["out_part"] + bo
    return out, attn


# revision 5
# speedup vs baseline: 3.3701x; 3.3701x over previous
"""Trainium2 Bass kernel for multi-head attention (B=4, T=S=1024, E=1024, H=16).

Sharding: 8 cores; core c handles batch b=c//2 and head-group g=c%2 (8 heads,
Megatron-style column split of Wq/Wk/Wv, row split of Wo). Each core returns
its 8 heads' attention probabilities and a partial out-projection; the host
sums the two partials per batch and adds the bias.

Per-core dataflow (Tile-scheduled on one NeuronCore):
  1. PE-transpose embeddings (float32r, bit-exact) -> embT [e, seq].
  2. Q^T / K^T (f32r) and V (bf16) projections.
  3. t-side, per (head, t-tile): scores = Q_h^T.T @ K_h^T (f32r, PSUM) with
     the merged mask added via a bf16 identity-lhsT matmul; ACT Exp with
     fused row-sum (accum_out); DVE reciprocal + normalize -> f32 attn out.
     No max-subtraction: scores are bounded for this problem (verified in
     test harness against the reference).
  4. T-side, per (head, s-tile): scores^T = K_h^T-as-lhsT @ Q_h^T (f32r)
     + mask^T via identity matmul; ACT Exp -> UNNORMALIZED E^T in bf16.
     This avoids transposing the softmax output entirely.
  5. PV: O' = V-chunks.T @ E^T (bf16, col-packed pairs). Normalization is
     applied after PV: O = O' * R, where R[p, t] = recip_{head(p)}[t] is
     materialized by broadcasting recip columns along the free dim (native
     per-partition broadcast) and PE-transposing 128x128 blocks.
  6. Out-projection from O^T chunks (f32r) against the Wo row-slice.
"""

import math

import numpy as np
import ml_dtypes

import concourse.bass as bass
import concourse.mybir as mybir
import concourse.tile as tile
from concourse import bacc, bass_utils

B, T, S, E = 4, 1024, 1024, 1024
H, D = 16, 64
HG, DG = 8, 512  # heads / head-dims per core
P = 128
N_CORES = 8
F32 = mybir.dt.float32
F32R = mybir.dt.float32r
BF16 = mybir.dt.bfloat16
AF = mybir.ActivationFunctionType

ET, TT, ST = E // P, T // P, S // P  # 8, 8, 8

ts = bass.ts

# toggled by test.py to capture a hardware profile
TRACE = False
LAST_RESULTS = None


def _build(nc):
    temb = nc.dram_tensor("temb", [T, E], F32R, kind="ExternalInput").ap()
    semb = nc.dram_tensor("semb", [S, E], F32R, kind="ExternalInput").ap()
    wq = nc.dram_tensor("wq", [E, DG], F32R, kind="ExternalInput").ap()
    wk = nc.dram_tensor("wk", [E, DG], F32R, kind="ExternalInput").ap()
    wv = nc.dram_tensor("wv", [E, DG], F32R, kind="ExternalInput").ap()
    wo = nc.dram_tensor("wo", [DG, E], F32R, kind="ExternalInput").ap()
    msk = nc.dram_tensor("msk", [T, S], BF16, kind="ExternalInput").ap()
    mskT = nc.dram_tensor("mskT", [S, T], BF16, kind="ExternalInput").ap()
    idf = nc.dram_tensor("idf", [P, P], F32R, kind="ExternalInput").ap()
    idb = nc.dram_tensor("idb", [P, P], BF16, kind="ExternalInput").ap()
    idf32 = nc.dram_tensor("idf32", [P, P], F32, kind="ExternalInput").ap()
    attn_out = nc.dram_tensor("attn_out", [HG, T, S], F32, kind="ExternalOutput").ap()
    out_part = nc.dram_tensor("out_part", [T, E], F32, kind="ExternalOutput").ap()

    with tile.TileContext(nc) as tc:
        with (
            tc.tile_pool(name="const", bufs=1) as constp,
            tc.tile_pool(name="persist", bufs=1) as persist,
        ):
            idf_sb = constp.tile([P, P], F32R, name="idf_sb")
            nc.sync.dma_start(idf_sb[:], idf)
            idb_sb = constp.tile([P, P], BF16, name="idb_sb")
            nc.sync.dma_start(idb_sb[:], idb)
            id32_sb = constp.tile([P, P], F32, name="id32_sb")
            nc.sync.dma_start(id32_sb[:], idf32)
            ones_t = constp.tile([P, 64], F32, name="ones_t")
            nc.gpsimd.memset(ones_t[:], 1.0)

            msk_sb = persist.tile([P, TT, S], BF16, name="msk_sb")  # [p, tt, s]
            nc.sync.dma_start(msk_sb[:], msk.rearrange("(tt p) s -> p tt s", p=P))
            mskT_sb = persist.tile([P, ST, T], BF16, name="mskT_sb")  # [p, st, t]
            nc.sync.dma_start(mskT_sb[:], mskT.rearrange("(st p) t -> p st t", p=P))
            wo_sb = persist.tile([P, DG // P, E], F32R, name="wo_sb")  # [p, chunk, e]
            nc.sync.dma_start(wo_sb[:], wo.rearrange("(c p) e -> p c e", p=P))

            qT_sb = persist.tile([P, DG // P, T], F32R, name="qT_sb")  # [p, dc, t]
            kT_sb = persist.tile([P, DG // P, S], F32R, name="kT_sb")
            v_sb = persist.tile([P, ST, DG], BF16, name="v_sb")  # [p, sc, d]
            oT_sb = persist.tile([P, DG // P, T], F32R, name="oT_sb")  # [p, dc, t]

            # ---------------- phase A: embedding transpose + projections ----
            with (
                tc.tile_pool(name="pa_emb", bufs=1) as pae,
                tc.tile_pool(name="pa_w", bufs=2) as paw,
                tc.tile_pool(name="pa_rows", bufs=1) as par,
                tc.tile_pool(name="pa_ps", bufs=2, space="PSUM") as pap,
                tc.tile_pool(name="pa_ps2", bufs=2, space="PSUM") as pap2,
            ):

                def transpose_emb(src, name):
                    embT = pae.tile([P, ET, T], F32R, tag="embT", name=name)
                    for t4 in range(TT // 4):
                        rows = []
                        for j in range(4):
                            row = par.tile([P, E], F32R, tag="emb_row", bufs=6,
                                           name=f"row_{name}_{t4}_{j}")
                            nc.sync.dma_start(row[:], src[ts(t4 * 4 + j, P), :])
                            rows.append(row)
                        for eo in range(ET):
                            ps = pap.tile([P, 4 * P], F32R, tag="tr_ps", name="tr_ps")
                            for j in range(4):
                                nc.tensor.transpose(
                                    ps[:, ts(j, P)], rows[j][:, ts(eo, P)], idf_sb[:]
                                )
                            nc.vector.tensor_copy(embT[:, eo, ts(t4, 4 * P)], ps[:])
                    return embT

                def load_w(src, name):
                    w_sb = paw.tile([P, ET, DG], F32R, tag="w", name=name)
                    nc.sync.dma_start(w_sb[:], src.rearrange("(eo p) d -> p eo d", p=P))
                    return w_sb

                # Q^T from target embedding
                tembT = transpose_emb(temb, "tembT")
                wq_sb = load_w(wq, "wq_sb")
                for dc in range(DG // P):
                    for th in range(T // 512):
                        ps = pap2.tile([P, 512], F32, tag="proj_ps", name="proj_ps")
                        for eo in range(ET):
                            nc.tensor.matmul(
                                ps[:],
                                wq_sb[:, eo, ts(dc, P)],
                                tembT[:, eo, ts(th, 512)],
                                start=(eo == 0),
                                stop=(eo == ET - 1),
                            )
                        nc.vector.tensor_copy(qT_sb[:, dc, ts(th, 512)], ps[:])

                # K^T and V from source embedding
                sembT = transpose_emb(semb, "sembT")
                wk_sb = load_w(wk, "wk_sb")
                for dc in range(DG // P):
                    for sh in range(S // 512):
                        ps = pap2.tile([P, 512], F32, tag="proj_ps", name="proj_ps")
                        for eo in range(ET):
                            nc.tensor.matmul(
                                ps[:],
                                wk_sb[:, eo, ts(dc, P)],
                                sembT[:, eo, ts(sh, 512)],
                                start=(eo == 0),
                                stop=(eo == ET - 1),
                            )
                        nc.vector.tensor_copy(kT_sb[:, dc, ts(sh, 512)], ps[:])

                wv_sb = load_w(wv, "wv_sb")
                for sc in range(ST):
                    ps = pap2.tile([P, DG], F32, tag="proj_ps", name="proj_ps")
                    for eo in range(ET):
                        nc.tensor.matmul(
                            ps[:],
                            sembT[:, eo, ts(sc, P)],
                            wv_sb[:, eo, :],
                            start=(eo == 0),
                            stop=(eo == ET - 1),
                        )
                    nc.vector.tensor_copy(v_sb[:, sc, :], ps[:])  # f32 -> bf16

            # ---------------- phase B: attention, head pairs ----
            with (
                tc.tile_pool(name="pb_sbuf", bufs=3) as pb,
                tc.tile_pool(name="pb_eT", bufs=2) as pet,
                tc.tile_pool(name="pb_rec", bufs=2) as prc,
                tc.tile_pool(name="pb_ps", bufs=1, space="PSUM") as pbp,
                tc.tile_pool(name="pb_mm512", bufs=1, space="PSUM") as pbm,
            ):
                for pair in range(HG // 2):
                    hA, hB = 2 * pair, 2 * pair + 1
                    dc = pair
                    recs = []
                    for hi, hp in ((0, 0), (1, 64)):
                        rec = prc.tile([P, TT], F32, tag=f"rec{hi}",
                                       name=f"rec_{pair}_{hi}")
                        recs.append(rec)
                    # ---- t-side: scores, softmax stats, f32 attn output ----
                    for tt in range(TT):
                        pss = []
                        for hi, hp in ((0, 0), (1, 64)):
                            ps = pbp.tile([P, S], F32, tag=f"big{hi}",
                                          name=f"score_{pair}_{hi}")
                            pss.append(ps)
                            for sc in range(S // 512):
                                nc.tensor.matmul(
                                    ps[:, ts(sc, 512)],
                                    qT_sb[hp : hp + 64, dc, ts(tt, P)],
                                    kT_sb[hp : hp + 64, dc, ts(sc, 512)],
                                    start=True,
                                    stop=False,
                                )
                            for sc in range(S // 512):
                                nc.tensor.matmul(
                                    ps[:, ts(sc, 512)],
                                    idb_sb[:],
                                    msk_sb[:, tt, ts(sc, 512)],
                                    start=False,
                                    stop=True,
                                )
                        for hi, hp in ((0, 0), (1, 64)):
                            h = 2 * pair + hi
                            ex = pb.tile([P, S], F32, tag="ex", name="ex")
                            sums = pb.tile([P, 1], F32, tag="sums", name="sums")
                            nc.scalar.activation(
                                ex[:], pss[hi][:], AF.Exp, accum_out=sums[:]
                            )
                            nc.vector.reciprocal(recs[hi][:, tt : tt + 1], sums[:])
                            af = pb.tile([P, S], F32, tag="af", name="af")
                            nc.vector.tensor_scalar_mul(
                                af[:], ex[:], recs[hi][:, tt : tt + 1]
                            )
                            nc.sync.dma_start(attn_out[h, ts(tt, P), :], af[:])

                    # ---- T-side: scores^T -> unnormalized E^T (bf16) ----
                    eTs = []
                    for hi, hp in ((0, 0), (1, 64)):
                        eT = pet.tile([P, ST, T], BF16, tag=f"eT{hi}",
                                      name=f"eT_{pair}_{hi}")
                        eTs.append(eT)
                    for st in range(ST):
                        psTs = []
                        for hi, hp in ((0, 0), (1, 64)):
                            psT = pbp.tile([P, T], F32, tag=f"big{hi}",
                                           name=f"scT_{pair}_{hi}")
                            psTs.append(psT)
                            for th in range(T // 512):
                                nc.tensor.matmul(
                                    psT[:, ts(th, 512)],
                                    kT_sb[hp : hp + 64, dc, ts(st, P)],
                                    qT_sb[hp : hp + 64, dc, ts(th, 512)],
                                    start=True,
                                    stop=False,
                                )
                            for th in range(T // 512):
                                nc.tensor.matmul(
                                    psT[:, ts(th, 512)],
                                    idb_sb[:],
                                    mskT_sb[:, st, ts(th, 512)],
                                    start=False,
                                    stop=True,
                                )
                        for hi in (0, 1):
                            nc.scalar.activation(
                                eTs[hi][:, st, :], psTs[hi][:], AF.Exp
                            )

                    # ---- PV (col-packed pair) + R normalization ----
                    for th in range(T // 512):
                        ops = []
                        for hi, hp in ((0, 0), (1, 64)):
                            op = pbm.tile([64, 512], F32, tag=f"op{hi}",
                                          name=f"op_{pair}_{hi}")
                            ops.append(op)
                        for sc in range(ST):
                            for hi, hp in ((0, 0), (1, 64)):
                                h = 2 * pair + hi
                                nc.tensor.matmul(
                                    ops[hi][:],
                                    v_sb[:, sc, ts(h, 64)],
                                    eTs[hi][:, sc, ts(th, 512)],
                                    start=(sc == 0),
                                    stop=(sc == ST - 1),
                                )
                        # R[p, tq] = recip_{head(p)}[th*512 + tq]
                        psR = pbm.tile([P, 512], F32, tag="psR", name="psR")
                        for j in range(4):
                            tt = th * 4 + j
                            rt = pb.tile([P, P], F32, tag="rt", name="rt")
                            for hi, hp in ((0, 0), (1, 64)):
                                nc.vector.tensor_scalar_mul(
                                    rt[:, hp : hp + 64],
                                    ones_t[:, 0:64],
                                    recs[hi][:, tt : tt + 1],
                                )
                            nc.tensor.transpose(psR[:, ts(j, P)], rt[:], id32_sb[:])
                        r_sb = pb.tile([P, 512], F32, tag="r_sb", name="r_sb")
                        nc.vector.tensor_copy(r_sb[:], psR[:])
                        for hi, hp in ((0, 0), (1, 64)):
                            nc.vector.tensor_mul(
                                oT_sb[hp : hp + 64, pair, ts(th, 512)],
                                ops[hi][:],
                                r_sb[hp : hp + 64, :],
                            )

                # ---------------- phase C: out projection ----
                for tt in range(TT):
                    for eh in range(E // 512):
                        ps = pbm.tile([P, 512], F32, tag="psR", name="out_ps")
                        for c in range(DG // P):
                            nc.tensor.matmul(
                                ps[:],
                                oT_sb[:, c, ts(tt, P)],
                                wo_sb[:, c, ts(eh, 512)],
                                start=(c == 0),
                                stop=(c == DG // P - 1),
                            )
                        ou = pb.tile([P, 512], F32, tag="ou", name="ou")
                        nc.scalar.copy(ou[:], ps[:])
                        nc.sync.dma_start(out_part[ts(tt, P), ts(eh, 512)], ou[:])

    return nc


_compiled = None


def _get_compiled():
    global _compiled
    if _compiled is None:
        nc = bacc.Bacc("TRN2", debug=False)
        _build(nc)
        nc.compile()
        _compiled = nc
    return _compiled


def kernel(**inputs):
    global LAST_RESULTS
    tgt = np.asarray(inputs["target_emb"], dtype=np.float32)
    src = np.asarray(inputs["source_emb"], dtype=np.float32)
    tpad = np.asarray(inputs["target_pad_mask"], dtype=np.float32)
    spad = np.asarray(inputs["source_pad_mask"], dtype=np.float32)
    amask = np.asarray(inputs["attention_mask"], dtype=np.float32)
    causal = int(np.asarray(inputs["is_casual"]))
    Wq = np.asarray(inputs["Wq"], dtype=np.float32)
    Wk = np.asarray(inputs["Wk"], dtype=np.float32)
    Wv = np.asarray(inputs["Wv"], dtype=np.float32)
    Wo = np.asarray(inputs["Wo"], dtype=np.float32)
    bo = np.asarray(inputs["bo"], dtype=np.float32)

    mask = tpad[:, :, None] * spad[:, None, :] * amask  # [B, T, S]
    if causal:
        mask = mask * np.tril(np.ones((T, S), dtype=np.float32))
    mask_bf = mask.astype(ml_dtypes.bfloat16)

    ident = np.eye(P, dtype=np.float32)
    ident_bf = np.eye(P, dtype=ml_dtypes.bfloat16)
    scale = np.float32(1.0 / math.sqrt(float(D)))

    nc = _get_compiled()
    in_maps = []
    for c in range(N_CORES):
        b, g = divmod(c, 2)
        in_maps.append(
            {
                "temb": np.ascontiguousarray(tgt[b]),
                "semb": np.ascontiguousarray(src[b]),
                "wq": np.ascontiguousarray(Wq[:, g * DG : (g + 1) * DG] * scale),
                "wk": np.ascontiguousarray(Wk[:, g * DG : (g + 1) * DG]),
                "wv": np.ascontiguousarray(Wv[:, g * DG : (g + 1) * DG]),
                "wo": np.ascontiguousarray(Wo[g * DG : (g + 1) * DG, :]),
                "msk": mask_bf[b],
                "mskT": np.ascontiguousarray(mask_bf[b].T),
                "idf": ident,
                "idb": ident_bf,
                "idf32": ident,
            }
        )

    results = bass_utils.run_bass_kernel_spmd(
        nc, in_maps, core_ids=list(range(N_CORES)), trace=TRACE
    )
    LAST_RESULTS = results
    res = results.results

    attn = np.empty((B, H, T, S), dtype=np.float32)
    out = np.empty((T, B, E), dtype=np.float32)
    for c in range(N_CORES):
        b, g = divmod(c, 2)
        attn[b, g * HG : (g + 1) * HG] = res[c]["attn_out"]
    for b in range(B):
        out[:, b, :] = res[2 * b]["out_part"] + res[2 * b + 1]["out_part"] + bo
    return out, attn
